# revision 7
# baseline (speedup 1.0000x reference)
"""CliqueGNN forward on 8 Trainium2 NeuronCores (Bass/Tile), edge-parallel.

Math (complete directed graph with self-loops, E = N^2, which is what the
model's _board_to_graph always builds — verified at runtime, with a host
fallback for arbitrary graphs):

  * deg == N for every node  =>  GCN norm == 1/N exactly; the aggregated
    message is identical for every node, so BatchNorm over nodes sees zero
    variance and collapses to its beta: x_{i+1} = relu(beta_gcn_i),
    broadcast over nodes.
  * concat(x[src], x[dst]) @ W == (x@W_top)[src] + (x@W_bot)[dst], so the
    two [E,256]@[256,128] MLPs reduce to tiny [384,...] matmuls plus
    per-edge broadcast adds:
        comb_i = ef_i @ (W_edge_i @ Wc_bot_i) + U_i[src] + V_i[dst] + cbias_i
    One [E,128]@[128,128] (layer1) / [E,3]@[3,128] (layer0, edge_emb chain
    folded) matmul per layer is the only per-edge GEMM.
  * BatchNorm over E needs global sum/sumsq -> one [128,2] AllReduce per
    layer; the masked softmax needs (max, sumexp) -> one [1,2] AllGather.

Sharding: edges split into 8 contiguous chunks of 18432 (sources 48c..48c+47
per core c). Each core streams its chunk in 48 blocks of [128ch x 384edge]
(channel-major layout), computes partial BN stats, exchanges them via
AllReduce, and emits exp(score - gmax)/Z for its edges plus the (replicated)
value head. The host only slices/transposes inputs and re-indexes the
upper-triangle output.
"""

import numpy as np

N = 384
H = 128
E = N * N
NCORES = 8
S = N // NCORES           # sources per core = 48
EC = E // NCORES          # edges per core = 18432
BLK = 384                 # edges per block (= one source's fan-out)
NB = EC // BLK            # 48 blocks per core
FB = BLK // 128           # score free-columns per block = 3
NF = EC // 128            # score free dim = 144
EPS = 1e-5
MASK_NEG = -87.0
RSQRT_MAGIC = 0x5F3759DF

_CACHE = {}


def _layout_params(params):
    """Flatten the nested param dict into the tensors the kernel consumes."""
    g = lambda a: np.ascontiguousarray(np.asarray(a, dtype=np.float32))
    col = lambda a: g(a).reshape(-1, 1)
    p = {}
    p["w_node"] = g(params["node_emb"][0])            # [1,128]
    p["b_node"] = col(params["node_emb"][1])          # [128,1]
    p["w_edge"] = g(params["edge_emb"][0])            # [3,128]
    p["b_edge"] = col(params["edge_emb"][1])          # [128,1]
    for i in range(2):
        wn = g(params[f"eb_node{i}"][0])              # [256,128]
        wc = g(params[f"eb_comb{i}"][0])              # [256,128]
        p[f"w1_{i}"] = np.ascontiguousarray(wn[:H])   # [128,128]
        p[f"w2_{i}"] = np.ascontiguousarray(wn[H:])   # [128,128]
        p[f"bn_{i}"] = col(params[f"eb_node{i}"][1])
        p[f"wct_{i}"] = np.ascontiguousarray(wc[:H])
        p[f"wcb_{i}"] = np.ascontiguousarray(wc[H:])
        p[f"bc_{i}"] = col(params[f"eb_comb{i}"][1])
        p[f"webe_{i}"] = g(params[f"eb_edge{i}"][0])  # [128,128]
        p[f"bebe_{i}"] = col(params[f"eb_edge{i}"][1])
        p[f"gam_{i}"] = col(params[f"eb_bn{i}"][0])
        p[f"bet_{i}"] = col(params[f"eb_bn{i}"][1])
        p[f"gbeta_{i}"] = col(params[f"gcn_bn{i}"][1])
    p["pol_w"] = g(params["policy"][0])               # [128,1]
    p["pol_b"] = g(params["policy"][1]).reshape(1, 1)
    p["v1_w"] = g(params["v1"][0])                    # [128,64]
    p["v1_b"] = col(params["v1"][1])                  # [64,1]
    p["v2_w"] = g(params["v2"][0])                    # [64,1]
    p["v2_b"] = g(params["v2"][1]).reshape(1, 1)
    return p


_PARAM_SHAPES = {
    "w_node": (1, H), "b_node": (H, 1), "w_edge": (3, H), "b_edge": (H, 1),
    "pol_w": (H, 1), "pol_b": (1, 1),
    "v1_w": (H, 64), "v1_b": (64, 1), "v2_w": (64, 1), "v2_b": (1, 1),
}
for _i in range(2):
    _PARAM_SHAPES.update({
        f"w1_{_i}": (H, H), f"w2_{_i}": (H, H), f"bn_{_i}": (H, 1),
        f"wct_{_i}": (H, H), f"wcb_{_i}": (H, H), f"bc_{_i}": (H, 1),
        f"webe_{_i}": (H, H), f"bebe_{_i}": (H, 1),
        f"gam_{_i}": (H, 1), f"bet_{_i}": (H, 1), f"gbeta_{_i}": (H, 1),
    })


def _build():
    import concourse.bass as bass
    import concourse.bacc as bacc
    import concourse.tile as tile
    from concourse import mybir

    f32 = mybir.dt.float32
    bf16 = mybir.dt.bfloat16
    i32 = mybir.dt.int32
    ADD = mybir.AluOpType.add
    MULT = mybir.AluOpType.mult
    ASR = mybir.AluOpType.arith_shift_right
    AF = mybir.ActivationFunctionType
    AX = mybir.AxisListType.X
    RG = [list(range(NCORES))]

    nc = bacc.Bacc("TRN2", target_bir_lowering=False, debug=False,
                   enable_asserts=True, num_devices=NCORES)

    def din(name, shape):
        return nc.dram_tensor(name, list(shape), f32, kind="ExternalInput").ap()

    attr = nc.dram_tensor("attr", [3, EC], mybir.dt.bfloat16,
                          kind="ExternalInput").ap()
    iota_src = din("iota_src", (1, S))
    iota_row = din("iota_row", (1, N))
    ones_row = din("ones_row", (1, N))
    ident_d = din("ident", (128, 128))
    maskb_d = din("maskb", (128, NF))
    P = {k: din(k, sh) for k, sh in _PARAM_SHAPES.items()}

    pol_out = nc.dram_tensor("pol", [128, NF], f32, kind="ExternalOutput").ap()
    val_out = nc.dram_tensor("val", [1, 1], f32, kind="ExternalOutput").ap()

    with tile.TileContext(nc) as tc:
        with tc.tile_pool(name="const", bufs=1) as const, \
             tc.tile_pool(name="stream_ef", bufs=NB) as pool_ef, \
             tc.tile_pool(name="stream_cb", bufs=NB) as pool_cb, \
             tc.tile_pool(name="work", bufs=3) as work, \
             tc.tile_pool(name="psm", bufs=4, space="PSUM") as psm, \
             tc.tile_pool(name="psaux", bufs=2, space="PSUM") as psaux, \
             tc.tile_pool(name="pssc", bufs=2, space="PSUM") as pssc, \
             tc.tile_pool(name="dram", bufs=1, space="DRAM") as dram:

            # ---------- helpers ----------
            def load(ap_dram, shape, tag):
                t = const.tile(list(shape), f32, tag=tag, name=tag)
                nc.sync.dma_start(t[:], ap_dram[:])
                return t

            def ps_aux(shape, tag="aux"):
                # single shared tag: all aux psum tiles rotate through the
                # same slots (each padded to one PSUM bank)
                return psaux.tile(list(shape), f32, tag="aux", name="aux")

            def to_sb(ps_t, shape, tag, bias=None, func=None, dt=f32):
                t = const.tile(list(shape), dt, tag=tag, name=tag)
                if func is None and bias is None:
                    nc.scalar.copy(t[:], ps_t[:])
                else:
                    nc.scalar.activation(t[:], ps_t[:],
                                         func or AF.Identity,
                                         bias=bias if bias is not None else 0.0)
                return t

            def mm(lhsT, rhs, m, n, tag="aux"):
                p = ps_aux((m, n), tag)
                nc.tensor.matmul(p[:], lhsT, rhs)
                return p

            def pe_T(in_sb, k, m, tag, dt=f32):
                """[k,m] -> [m,k] via PE transpose; returns SBUF tile."""
                p = ps_aux((m, k), "auxT")
                nc.tensor.matmul(p[:], in_sb, ident_sb[:k, :k], is_transpose=True)
                return to_sb(p, (m, k), tag, dt=dt)

            def colbrd(v11, tag):
                """[1,1] -> [128,1] broadcast column."""
                p = ps_aux((128, 1), "auxB")
                nc.tensor.matmul(p[:], ones_sb[:, :128], v11[:])
                return to_sb(p, (128, 1), tag)

            def small(shape, tag, dt=f32):
                return const.tile(list(shape), dt, tag=tag, name=tag)

            # ---------- constants ----------
            ident_sb = load(ident_d, (128, 128), "ident")
            ones_sb = load(ones_row, (1, N), "ones")
            iota_sb = load(iota_row, (1, N), "iota")
            iotas_sb = load(iota_src, (1, S), "iotas")
            maskb_sb = load(maskb_d, (128, NF), "maskb")
            W = {k: load(P[k], _PARAM_SHAPES[k], k) for k in _PARAM_SHAPES}

            # ---------- node features, layer 0 ----------
            x_ps = mm(W["w_node"][:], iota_sb[:], 128, N, "x0")
            x_T = to_sb(x_ps, (128, N), "x0T", bias=W["b_node"][:])
            xs_ps = mm(W["w_node"][:], iotas_sb[:], 128, S, "xs0")
            xs_T = to_sb(xs_ps, (128, S), "xs0T", bias=W["b_node"][:])

            ef_tiles = None
            for i in range(2):
                # ---------- per-layer precompute (tiny) ----------
                # cbias chain
                if i == 0:
                    c1p = mm(W["webe_0"][:], W["b_edge"][:], 128, 1, "c1")
                    c1 = to_sb(c1p, (128, 1), "c1sb", bias=W["bebe_0"][:])
                    c2p = mm(W["wcb_0"][:], c1[:], 128, 1, "c2")
                else:
                    c2p = mm(W["wcb_1"][:], W["bebe_1"][:], 128, 1, "c2")
                c3p = mm(W[f"wct_{i}"][:], W[f"bn_{i}"][:], 128, 1, "c3")
                c2sb = to_sb(c2p, (128, 1), f"c2sb_{i}", bias=W[f"bc_{i}"][:])
                cbias = small((128, 1), f"cbias_{i}")
                nc.vector.tensor_tensor(cbias[:], c2sb[:], c3p[:], ADD)

                # fused edge-weight chain
                if i == 0:
                    WeT = pe_T(W["w_edge"][:], 3, 128, "WeT")          # [128,3]
                    p1 = mm(W["webe_0"][:], WeT[:], 128, 3, "p1")
                    p1sb = to_sb(p1, (128, 3), "p1sb")
                    p2 = mm(W["wcb_0"][:], p1sb[:], 128, 3, "p2")
                    p2sb = to_sb(p2, (128, 3), "p2sb")
                    Wmain = pe_T(p2sb[:], 128, 3, "Wchain0", dt=bf16)  # [3,128]
                else:
                    WbT = pe_T(W["webe_1"][:], 128, 128, "WbT")
                    t1 = mm(W["wcb_1"][:], WbT[:], 128, 128, "t1")
                    t1sb = to_sb(t1, (128, 128), "t1sb")
                    Wmain = pe_T(t1sb[:], 128, 128, "Wcomb1", dt=bf16)  # [128,128]

                # U (core's sources) and V (all nodes)
                tu = to_sb(mm(W[f"w1_{i}"][:], xs_T[:], 128, S, "tu"),
                           (128, S), f"tu_{i}")
                Up = to_sb(mm(W[f"wct_{i}"][:], tu[:], 128, S, "up"),
                           (128, S), f"Up_{i}", bias=cbias[:])
                tv = to_sb(mm(W[f"w2_{i}"][:], x_T[:], 128, N, "tv"),
                           (128, N), f"tv_{i}")
                V = to_sb(mm(W[f"wct_{i}"][:], tv[:], 128, N, "vv"),
                          (128, N), f"V_{i}")

                # ---------- per-edge stream ----------
                sumP = small((128, NB), f"sumP_{i}")
                sqP = small((128, NB), f"sqP_{i}")
                comb_tiles = []
                for r in range(NB):
                    pblk = psm.tile([128, BLK], f32, tag="mmblk", name="mmblk")
                    if i == 0:
                        a_t = work.tile([3, BLK], bf16, tag="attrblk", name="attrblk")
                        nc.sync.dma_start(a_t[:], attr[:, r * BLK:(r + 1) * BLK])
                        nc.tensor.matmul(pblk[:], Wmain[:], a_t[:])
                    else:
                        nc.tensor.matmul(pblk[:], Wmain[:], ef_tiles[r][:])
                    cb_t = pool_cb.tile([128, BLK], f32, tag="comb", name="comb")
                    nc.vector.tensor_scalar(cb_t[:], pblk[:], Up[:, r:r + 1], None,
                                            op0=ADD, op1=ADD,
                                            accum_out=sumP[:, r:r + 1])
                    nc.vector.tensor_tensor(cb_t[:], cb_t[:], V[:], ADD)
                    scr = work.tile([128, BLK], f32, tag="sqscr", name="sqscr")
                    nc.scalar.activation(scr[:], cb_t[:], AF.Square,
                                         accum_out=sqP[:, r:r + 1])
                    comb_tiles.append(cb_t)

                # ---------- global BN stats ----------
                s1 = small((128, 1), f"s1_{i}")
                nc.vector.reduce_sum(s1[:], sumP[:], axis=AX)
                vr = small((128, 1), f"vr_{i}")
                nc.vector.reduce_sum(vr[:], V[:], axis=AX)
                stats = small((128, 2), f"stats_{i}")
                nc.vector.tensor_scalar(stats[:, 0:1], vr[:], float(NB), s1[:],
                                        op0=MULT, op1=ADD)
                nc.vector.reduce_sum(stats[:, 1:2], sqP[:], axis=AX)

                cc_in = dram.tile([128, 2], f32, tag=f"ccin_{i}", name=f"ccin_{i}")
                cc_out = dram.tile([128, 2], f32, tag=f"ccout_{i}", name=f"ccout_{i}")
                nc.sync.dma_start(cc_in[:], stats[:])
                nc.gpsimd.collective_compute(
                    "AllReduce", ADD, replica_groups=RG,
                    ins=[cc_in.opt()], outs=[cc_out.opt()])
                gst = small((128, 2), f"gst_{i}")
                nc.sync.dma_start(gst[:], cc_out[:])

                mu = small((128, 1), f"mu_{i}")
                nc.vector.tensor_scalar(mu[:], gst[:, 0:1], 1.0 / E, None, op0=MULT)
                ex2 = small((128, 1), f"ex2_{i}")
                nc.vector.tensor_scalar(ex2[:], gst[:, 1:2], 1.0 / E, None, op0=MULT)
                mu2 = small((128, 1), f"mu2_{i}")
                nc.vector.tensor_tensor(mu2[:], mu[:], mu[:], MULT)
                veps = small((128, 1), f"veps_{i}")
                nc.vector.tensor_scalar(veps[:], mu2[:], -1.0, EPS, op0=MULT, op1=ADD)
                nc.vector.tensor_tensor(veps[:], veps[:], ex2[:], ADD)

                # rsqrt(veps) on DVE: bit-trick + 3 Newton steps
                y = small((128, 1), f"y_{i}")
                yi = y[:].bitcast(i32)
                nc.vector.tensor_scalar(yi, veps[:].bitcast(i32), 1, None, op0=ASR)
                nc.vector.tensor_scalar(yi, yi, -1, RSQRT_MAGIC, op0=MULT, op1=ADD)
                nr = small((128, 1), f"nr_{i}")
                for _ in range(3):
                    nc.vector.tensor_tensor(nr[:], y[:], y[:], MULT)
                    nc.vector.tensor_tensor(nr[:], nr[:], veps[:], MULT)
                    nc.vector.tensor_scalar(nr[:], nr[:], -0.5, 1.5, op0=MULT, op1=ADD)
                    nc.vector.tensor_tensor(y[:], y[:], nr[:], MULT)

                scale = small((128, 1), f"scale_{i}")
                nc.vector.tensor_tensor(scale[:], y[:], W[f"gam_{i}"][:], MULT)
                msc = small((128, 1), f"msc_{i}")
                nc.vector.tensor_tensor(msc[:], mu[:], scale[:], MULT)
                beff = small((128, 1), f"beff_{i}")
                nc.vector.tensor_scalar(beff[:], msc[:], -1.0, W[f"bet_{i}"][:],
                                        op0=MULT, op1=ADD)

                # ---------- normalize + relu ----------
                new_ef = []
                for r in range(NB):
                    e_t = pool_ef.tile([128, BLK], bf16, tag="ef", name="ef")
                    nc.scalar.activation(e_t[:], comb_tiles[r][:], AF.Relu,
                                         bias=beff[:], scale=scale[:])
                    new_ef.append(e_t)
                ef_tiles = new_ef

                # ---------- node features for next layer ----------
                xc = small((128, 1), f"xcol_{i}")
                nc.scalar.activation(xc[:], W[f"gbeta_{i}"][:], AF.Relu)
                if i == 0:
                    xrow = pe_T(xc[:], 128, 1, "xrow0")                 # [1,128]
                    xp = ps_aux((128, N), "xbrd")
                    nc.tensor.matmul(xp[:], xrow[:], ones_sb[:])
                    x_T = to_sb(xp, (128, N), "x1T")
                    xs_T = x_T[:, :S]  # columns are identical

            # ---------- policy head ----------
            # score column F (= edges F*128..F*128+127) is ef_slice.T @ w:
            # one [128,1] psum column per matmul, 144 columns in one bank.
            scores = small((128, NF), "scores")
            polw_bf = small((128, 1), "polw_bf", dt=bf16)
            nc.scalar.copy(polw_bf[:], W["pol_w"][:])
            p_sc = pssc.tile([128, NF], f32, tag="sc", name="sc")
            for r in range(NB):
                for cch in range(FB):
                    F = r * FB + cch
                    nc.tensor.matmul(
                        p_sc[:, F:F + 1],
                        ef_tiles[r][:, cch * 128:(cch + 1) * 128],
                        polw_bf[:])
            nc.vector.tensor_tensor(scores[:], p_sc[:], maskb_sb[:], ADD)

            mx = small((128, 1), "mx")
            nc.vector.reduce_max(mx[:], scores[:], axis=AX)
            mxT = pe_T(mx[:], 128, 1, "mxT")                            # [1,128]
            mloc = small((1, 1), "mloc")
            nc.vector.reduce_max(mloc[:], mxT[:], axis=AX)

            pm = small((1, 1), "pm")
            nc.vector.tensor_scalar(pm[:], mloc[:], -1.0, W["pol_b"][:],
                                    op0=MULT, op1=ADD)
            pmcol = colbrd(pm, "pmcol")
            exs = small((128, NF), "exs")
            zparts = small((128, 1), "zparts")
            nc.scalar.activation(exs[:], scores[:], AF.Exp, bias=pmcol[:],
                                 accum_out=zparts[:])
            zT = pe_T(zparts[:], 128, 1, "zT")
            zloc = small((1, 1), "zloc")
            nc.vector.reduce_sum(zloc[:], zT[:], axis=AX)

            pair = small((1, 2), "pair")
            nc.vector.tensor_copy(pair[:, 0:1], mloc[:])
            nc.vector.tensor_copy(pair[:, 1:2], zloc[:])
            ag_in = dram.tile([1, 2], f32, tag="agin", name="agin")
            ag_out = dram.tile([NCORES, 2], f32, tag="agout", name="agout")
            nc.sync.dma_start(ag_in[:], pair[:])
            nc.gpsimd.collective_compute(
                "AllGather", mybir.AluOpType.bypass, replica_groups=RG,
                ins=[ag_in.opt()], outs=[ag_out.opt()])
            ms8 = small((1, NCORES), "ms8")
            zs8 = small((1, NCORES), "zs8")
            nc.sync.dma_start(ms8[:], ag_out[:, 0:1].rearrange("c a -> a c"))
            nc.sync.dma_start(zs8[:], ag_out[:, 1:2].rearrange("c a -> a c"))

            gmax = small((1, 1), "gmax")
            nc.vector.reduce_max(gmax[:], ms8[:], axis=AX)
            ng = small((1, 1), "ng")
            nc.vector.tensor_scalar(ng[:], gmax[:], -1.0, None, op0=MULT)
            e8 = small((1, NCORES), "e8")
            nc.scalar.activation(e8[:], ms8[:], AF.Exp, bias=ng[:])
            nc.vector.tensor_tensor(e8[:], e8[:], zs8[:], MULT)
            zg = small((1, 1), "zg")
            nc.vector.reduce_sum(zg[:], e8[:], axis=AX)

            eloc = small((1, 1), "eloc")
            nc.scalar.activation(eloc[:], mloc[:], AF.Exp, bias=ng[:])
            izg = small((1, 1), "izg")
            nc.vector.reciprocal(izg[:], zg[:])
            alpha = small((1, 1), "alpha")
            nc.vector.tensor_tensor(alpha[:], eloc[:], izg[:], MULT)
            acol = colbrd(alpha, "acol")

            polsb = small((128, NF), "polsb")
            nc.vector.tensor_scalar(polsb[:], exs[:], acol[:], None, op0=MULT)
            nc.sync.dma_start(pol_out[:], polsb[:])

            # ---------- value head ----------
            x2c = small((128, 1), "x2c")
            nc.scalar.activation(x2c[:], W["gbeta_1"][:], AF.Relu)
            hps = ps_aux((64, 1), "hps")
            nc.tensor.matmul(hps[:], W["v1_w"][:], x2c[:])
            hsb = to_sb(hps, (64, 1), "hsb", bias=W["v1_b"][:], func=AF.Relu)
            vps = ps_aux((1, 1), "vps")
            nc.tensor.matmul(vps[:], W["v2_w"][:], hsb[:])
            vsb = to_sb(vps, (1, 1), "vsb", bias=W["v2_b"][:], func=AF.Tanh)
            nc.sync.dma_start(val_out[:], vsb[:])

    nc.compile()
    return nc


def _get_nc():
    if "nc" not in _CACHE:
        _CACHE["nc"] = _build()
    return _CACHE["nc"]


def _host_inputs(edge_attr, params):
    """Per-core in_maps for the SPMD kernel."""
    p = _layout_params(params)
    ea = np.asarray(edge_attr, dtype=np.float32)

    iota_row = np.arange(N, dtype=np.float32).reshape(1, N)
    ones_row = np.ones((1, N), np.float32)
    ident = np.eye(128, dtype=np.float32)

    import ml_dtypes
    bf16 = np.dtype(ml_dtypes.bfloat16)
    in_maps = []
    for c in range(NCORES):
        attr_T = np.ascontiguousarray(ea[c * EC:(c + 1) * EC].T).astype(bf16)
        g = c * EC + (np.arange(NF)[None, :] * 128 + np.arange(128)[:, None])
        s = g // N
        d = g % N
        maskb = np.where(s < d, 0.0, MASK_NEG).astype(np.float32)
        m = {
            "attr": attr_T,
            "iota_src": np.arange(c * S, (c + 1) * S, dtype=np.float32).reshape(1, S),
            "iota_row": iota_row,
            "ones_row": ones_row,
            "ident": ident,
            "maskb": maskb,
        }
        m.update(p)
        in_maps.append(m)
    return in_maps


def _is_complete_graph(edge_index):
    ei = np.asarray(edge_index)
    if ei.shape != (2, E):
        return False
    src = np.repeat(np.arange(N, dtype=ei.dtype), N)
    dst = np.tile(np.arange(N, dtype=ei.dtype), N)
    return bool(np.array_equal(ei[0], src) and np.array_equal(ei[1], dst))


def _fallback(edge_index, edge_attr, params):
    """Host numpy reference for unexpected graph structure (not the fast path)."""
    f32 = np.float32
    src = np.asarray(edge_index[0]); dst = np.asarray(edge_index[1])
    ea = np.asarray(edge_attr, f32)
    p = {k: (np.asarray(v[0], f32), np.asarray(v[1], f32))
         for k, v in params.items()}

    def bn(h, gamma, beta):
        mu = h.mean(0); var = h.var(0)
        return (h - mu) / np.sqrt(var + EPS) * gamma + beta

    x = np.arange(N, dtype=f32)[:, None] @ p["node_emb"][0] + p["node_emb"][1]
    ef = ea @ p["edge_emb"][0] + p["edge_emb"][1]
    deg = np.zeros(N, f32); np.add.at(deg, dst, 1.0)
    dinv = np.where(deg > 0, 1.0 / np.sqrt(np.maximum(deg, 1.0)), 0.0)
    norm = (dinv[src] * dinv[dst])[:, None]
    for i in range(2):
        w, b = p[f"gcn{i}"]
        agg = np.zeros((N, H), f32)
        np.add.at(agg, dst, (x @ w)[src] * norm)
        agg += b
        x_new = np.maximum(bn(agg, *p[f"gcn_bn{i}"]), 0)
        nf = np.concatenate([x[src], x[dst]], 1) @ p[f"eb_node{i}"][0] + p[f"eb_node{i}"][1]
        epf = ef @ p[f"eb_edge{i}"][0] + p[f"eb_edge{i}"][1]
        comb = np.concatenate([nf, epf], 1) @ p[f"eb_comb{i}"][0] + p[f"eb_comb{i}"][1]
        ef = np.maximum(bn(comb, *p[f"eb_bn{i}"]), 0)
        x = x_new
    scores = (ef @ p["policy"][0] + p["policy"][1])[:, 0]
    Pn = N * (N - 1) // 2
    valid = src < dst
    tri = src * (2 * N - src - 1) // 2 + (dst - src - 1)
    logits = np.zeros(Pn + 1, f32)
    logits[np.where(valid, tri, Pn)] = scores
    logits = logits[:Pn]
    ex = np.exp(logits - logits.max())
    policy = (ex / ex.sum()).astype(f32)
    h = np.maximum(x.mean(0, keepdims=True) @ p["v1"][0] + p["v1"][1], 0)
    value = np.tanh(h @ p["v2"][0] + p["v2"][1]).astype(f32)
    return policy, value


def kernel(edge_index, edge_attr, params):
    if not _is_complete_graph(edge_index):
        return _fallback(edge_index, edge_attr, params)

    from concourse import bass_utils

    nc = _get_nc()
    in_maps = _host_inputs(edge_attr, params)
    res = bass_utils.run_bass_kernel_spmd(nc, in_maps,
                                          core_ids=list(range(NCORES)))

    vec = np.concatenate([np.ascontiguousarray(res.results[c]["pol"].T).ravel()
                          for c in range(NCORES)])
    src = np.repeat(np.arange(N), N)
    dst = np.tile(np.arange(N), N)
    policy = vec[src < dst].astype(np.float32)
    value = np.asarray(res.results[0]["val"], np.float32).reshape(1, 1)
    return policy, value
